# revision 40
# baseline (speedup 1.0000x reference)
"""Trainium2 Bass kernel for nn_AttentionBlock (sparse attention block), v2.

Reference computation (B=4, C=512, T=2048, H=8 heads, 32 GN groups):
    xn  = GroupNorm(x) * gn_w + gn_b
    qkv = qkv_w @ xn + qkv_b            (1x1 conv)
    q,k,v = split(reshape(qkv, [B*H, 192, T])) ; each += pos
    S   = (q*s)^T (k*s),  s = ch^-0.25  => scale 1/8 on logits
    S[mask keys] = -1e9 ; P = softmax(S, axis=keys)
    h   = P @ v ; out = x + proj_w @ h + proj_b

Mask quirk (faithful to the reference): jnp.tile(mask,(H,1,1)) tiles
head-major, so attention row n = b*H + h uses mask[n % B] = mask[h % 4].

Sharding: 8 cores = (batch b, query-half j).  Each core computes
out[b][:, j*1024:(j+1)*1024] completely; host concatenates.  No collectives.

Sparsity: host compacts the key axis per mask-group m = h%4 with
keep_m = ~mask[m] (about half of T), padded to a multiple of 128.  Padded
key rows get an exp-bias of -1e9 so their softmax weight is exactly 0.

v2 device layout (head pair (a,b) = slots (2m, 2m+1) = true heads (m, m+4)):
  - S^T [keys, queries] computed per (m, s-chunk, tb): two K=64 matmuls
    row-tiled at array rows 0:64 / 64:128 into one [128,1024] PSUM tile
    (head a cols 0:512, head b cols 512:1024) -> ONE exp per chunk.
  - PV col-tiled: head a -> O2[0:64], head b -> O2[64:128] of a single
    [128,TH] accumulator; softmax denominators from two M=1 matmuls
    (ones lhsT) into den[0] / den[64], re-streaming the same ex chunk.
  - normalize: reciprocal_approx_fast on the two den rows, then ONE
    K=65 f32r select-matmul broadcasts 1/l_a to partitions 0:64 and
    1/l_b to 64:128; one DVE multiply forms h2 [128, TH] per pair.
  - proj contracts pairs with K=128 (wp row-pairs vs h2 tiles).
  - Emission is software-pipelined: PV/den lag S/exp by several chunks,
    v-phase and qk of later pairs interleave as PE filler, so the PE
    never idles long enough for the HAM clock gate to re-throttle.
PSUM budget (8 banks): mm pool 2x[128,1024] (4) + O2 (2) + den (2).
"""

import numpy as np
import ml_dtypes

B, C, T, H = 4, 512, 2048, 8
CH = C // H          # 64 channels per head
TH = T // 2          # 1024 query columns per core
P = 128
NUM_GROUPS = 32
GS = C // NUM_GROUPS  # 16 channels per group
EPS = 1e-5
BF16 = ml_dtypes.bfloat16
NMG = 4              # mask groups (= B); group m covers heads m and m+4
PERM = [0, 4, 1, 5, 2, 6, 3, 7]  # slot s holds true head PERM[s]

_graph_cache = {}


def _build(nkv):
    """Build the Bass graph for one core (SPMD: all 8 cores run this graph)."""
    import concourse.tile as tile
    from concourse import bacc, mybir

    f32 = mybir.dt.float32
    f32r = mybir.dt.float32r
    bf16 = mybir.dt.bfloat16
    AF = mybir.ActivationFunctionType
    OP = mybir.AluOpType

    sc_n = nkv // P  # number of 128-wide key chunks

    nc = bacc.Bacc("TRN2")

    # ---- DRAM parameters (per-core shards; host fills these) ----
    d_xq = nc.dram_tensor("x_q", [C, TH], bf16, kind="ExternalInput")
    d_xkv = nc.dram_tensor("x_kv", [NMG, C, nkv], bf16, kind="ExternalInput")
    d_xres = nc.dram_tensor("x_res", [C, TH], bf16, kind="ExternalInput")
    d_posq = nc.dram_tensor("pos_q", [C, TH], bf16, kind="ExternalInput")
    d_poskv = nc.dram_tensor("pos_kv", [NMG, P, nkv], bf16, kind="ExternalInput")
    d_posT = nc.dram_tensor("posT_kv", [NMG, P, nkv], bf16, kind="ExternalInput")
    d_wqkvT = nc.dram_tensor("wqkvT", [C, 3 * C], bf16, kind="ExternalInput")
    d_wpT = nc.dram_tensor("wpT", [C, C], bf16, kind="ExternalInput")
    d_pad = nc.dram_tensor("pad_bias", [NMG, P, nkv // P], f32,
                            kind="ExternalInput")
    d_out = nc.dram_tensor("out", [C, TH], bf16, kind="ExternalOutput")
    DBG = False
    if DBG:
        d_den = nc.dram_tensor("dbg_den", [NMG, 2, TH], f32,
                               kind="ExternalOutput")
        d_rc = nc.dram_tensor("dbg_rc", [NMG, 2, TH], f32,
                              kind="ExternalOutput")
        d_h2 = nc.dram_tensor("dbg_h2", [NMG, P, TH], f32,
                              kind="ExternalOutput")
        d_o2 = nc.dram_tensor("dbg_o2", [NMG, P, TH], f32,
                              kind="ExternalOutput")
        d_ex0 = nc.dram_tensor("dbg_ex0", [P, TH], f32, kind="ExternalOutput")

    with tile.TileContext(nc) as tc, \
         tc.tile_pool(name="persist", bufs=1) as pers:

        def ptile(shape, dt_, name):
            return pers.tile(shape, dt_, tag=name, name=name)

        # --- tiny exp to pull the ACT table load off the critical path ---
        warm_in = ptile([1, 1], f32, "warm_in")
        warm_out = ptile([1, 1], f32, "warm_out")
        nc.vector.memset(warm_in, 0.0)
        nc.scalar.activation(out=warm_out, in_=warm_in, func=AF.Exp)

        # --- persistent SBUF arrays ---
        xq = [ptile([P, TH], bf16, f"xq{i}") for i in range(4)]
        xkv = [[ptile([P, nkv], bf16, f"xkv{m}_{i}") for i in range(4)]
               for m in range(NMG)]
        wq = [ptile([P, 3 * C], bf16, f"wq{i}") for i in range(4)]
        wp2 = [ptile([P, C], bf16, f"wp{m}") for m in range(NMG)]
        posq = [ptile([P, TH], bf16, f"posq{i}") for i in range(4)]
        poskv = [ptile([P, nkv], bf16, f"poskv{m}") for m in range(NMG)]
        q_sb = [ptile([P, TH], bf16, f"q{m}") for m in range(NMG)]
        kblk = [(st, min(512, nkv - st)) for st in range(0, nkv, 512)]
        k_sb = [[ptile([P, w], bf16, f"k{m}_{bi}")
                 for bi, (st, w) in enumerate(kblk)] for m in range(NMG)]
        vhat = [[ptile([P, P], bf16, f"vhat{m}_{s}") for s in range(sc_n)]
                for m in range(NMG)]
        h2 = [ptile([P, TH], bf16, f"h2_{m}") for m in range(NMG)]
        xres = [ptile([P, TH], bf16, f"xres{i}") for i in range(4)]
        pad_sb = [ptile([P, sc_n], f32, f"pad{m}") for m in range(NMG)]
        # select matrix for the paired reciprocal broadcast (K=65 matmul):
        # row 0 -> output partitions 0:64, row 64 -> partitions 64:128.
        sel = ptile([65, P], f32, "sel")
        ones128 = ptile([P, 1], bf16, "ones128")
        # per-pair reciprocal rows (rows 1:64 stay 1.0 = harmless filler)
        rc = [ptile([65, TH], f32, f"rc{m}") for m in range(NMG)]
        bc_sb = [ptile([P, TH], bf16, f"bcs{m}") for m in range(NMG)]
        psb_t = [ptile([P, TH], f32, f"psb{ci}") for ci in range(4)]

        wrm = ptile([P, 512], bf16, "wrm")

        def emit_pe_warmup():
            nc.vector.memset(ones128, 1.0)
            nc.vector.memset(wrm, 0.0)
            wps = None
            for w_i in range(36):
                if w_i % 12 == 0:
                    wps = mmp.tile([1, 512], f32, tag="mm", name=f"wps{w_i}")
                nc.tensor.matmul(wps, ones128, wrm, start=True, stop=True)

        def emit_norm_memsets():
            nc.vector.memset(sel, 0.0)
            nc.vector.memset(sel[0:1, 0:CH], 1.0)
            nc.vector.memset(sel[64:65, CH:P], 1.0)
            nc.vector.memset(ones128, 1.0)
            for m in range(NMG):
                nc.vector.memset(rc[m], 1.0)

        # --- input DMAs, grouped by consuming pair and spread over idle
        # engine queues (each trigger serializes ~0.7us on its queue) ---
        posvT = [ptile([P, sc_n * P], bf16, f"pvT{m}") for m in range(NMG)]

        def emit_pair_dmas(q, m):
            for i in range(4):
                q.dma_start(xkv[m][i], d_xkv[m, i * P:(i + 1) * P, :])
            q.dma_start(poskv[m], d_poskv[m, :, :])
            q.dma_start(pad_sb[m], d_pad[m, :, :])
            q.dma_start(posvT[m], d_posT[m, :, :])

        # critical prefix balanced ~1.7MB/queue: exactly the k(0)/q(0)/
        # v(0) operand set, (xkv0[i], wq[i]) adjacent per queue
        nc.sync.dma_start(xkv[0][0], d_xkv[0, 0:P, :])
        nc.sync.dma_start(wq[0], d_wqkvT[0:P, :])
        nc.sync.dma_start(poskv[0], d_poskv[0, :, :])
        nc.sync.dma_start(posq[0], d_posq[0:P, :])
        nc.sync.dma_start(xq[2], d_xq[2 * P:3 * P, :])
        nc.sync.dma_start(posq[1], d_posq[P:2 * P, :])
        nc.scalar.dma_start(xkv[0][1], d_xkv[0, P:2 * P, :])
        nc.scalar.dma_start(wq[1], d_wqkvT[P:2 * P, :])
        nc.scalar.dma_start(xq[0], d_xq[0:P, :])
        nc.scalar.dma_start(pad_sb[0], d_pad[0, :, :])
        nc.scalar.dma_start(xq[3], d_xq[3 * P:4 * P, :])
        nc.scalar.dma_start(posvT[0], d_posT[0, :, :])
        nc.scalar.dma_start(posq[2], d_posq[2 * P:3 * P, :])
        nc.gpsimd.dma_start(xkv[0][2], d_xkv[0, 2 * P:3 * P, :])
        nc.gpsimd.dma_start(wq[2], d_wqkvT[2 * P:3 * P, :])
        nc.gpsimd.dma_start(xkv[0][3], d_xkv[0, 3 * P:4 * P, :])
        nc.gpsimd.dma_start(wq[3], d_wqkvT[3 * P:4 * P, :])
        nc.gpsimd.dma_start(xq[1], d_xq[P:2 * P, :])
        nc.gpsimd.dma_start(posq[3], d_posq[3 * P:4 * P, :])
        emit_pair_dmas(nc.scalar, 1)
        for i in range(4):
            nc.sync.dma_start(xres[i], d_xres[i * P:(i + 1) * P, :])
        emit_pair_dmas(nc.sync, 2)
        emit_pair_dmas(nc.gpsimd, 3)
        for m in range(NMG):
            nc.scalar.dma_start(wp2[m], d_wpT[m * P:(m + 1) * P, :])

        with tc.tile_pool(name="mm", bufs=2, space="PSUM") as mmp, \
             tc.tile_pool(name="o2p", bufs=1, space="PSUM") as o2p, \
             tc.tile_pool(name="denp", bufs=1, space="PSUM") as denp, \
             tc.tile_pool(name="exps", bufs=12) as epl, \
             tc.tile_pool(name="misc", bufs=2) as msc:

            def emit_v_chunk(m, s, pool=None, tag="mm"):
                # vhat[m][s] [128 keys, 128] = (xn_kv^T @ Wv_pair) + posT
                pv = (pool or mmp).tile([P, P], f32, tag=tag,
                                        name=f"psv{m}_{s}")
                for i in range(4):
                    nc.tensor.matmul(
                        pv, xkv[m][i][:, s * P:(s + 1) * P],
                        wq[i][:, 2 * C + m * P:2 * C + (m + 1) * P],
                        start=(i == 0), stop=(i == 3))
                nc.vector.tensor_add(vhat[m][s], pv,
                                     posvT[m][:, s * P:(s + 1) * P])

            def emit_q(m):
                # q channels (slot order) [128*m, 128*m+128)
                pq = mmp.tile([P, TH], f32, tag="mm", name=f"psq{m}")
                for tb in range(2):
                    for i in range(4):
                        nc.tensor.matmul(
                            pq[:, tb * 512:(tb + 1) * 512],
                            wq[i][:, m * P:(m + 1) * P],
                            xq[i][:, tb * 512:(tb + 1) * 512],
                            start=(i == 0), stop=(i == 3))
                nc.vector.tensor_add(q_sb[m], pq, posq[m])

            def emit_k_block(m, bi):
                for bj, (st, w) in enumerate(kblk):
                    if bj != bi:
                        continue
                    pk = mmp.tile([P, 512], f32, tag="mm", name=f"psk{m}_{st}")
                    for i in range(4):
                        nc.tensor.matmul(
                            pk[:, 0:w],
                            wq[i][:, C + m * P:C + (m + 1) * P],
                            xkv[m][i][:, st:st + w],
                            start=(i == 0), stop=(i == 3))
                    nc.vector.tensor_add(
                        k_sb[m][bi], pk[:, 0:w],
                        poskv[m][:, st:st + w])

            def emit_k(m):
                for bi in range(len(kblk)):
                    emit_k_block(m, bi)

            def emit_qk(m):
                emit_q(m)
                emit_k(m)

            # ---------- attention pipeline ----------
            def emit_s_exp(m, s, tb):
                """S^T chunk for both heads + one fused exp."""
                s2 = mmp.tile([P, TH], f32, tag="mm", name=f"s2_{m}_{s}_{tb}")
                qs = slice(tb * 512, (tb + 1) * 512)
                bi, off = (s * P) // 512, (s * P) % 512
                kt = k_sb[m][bi]
                nc.tensor.matmul(
                    s2[:, 0:512], kt[0:CH, off:off + P], q_sb[m][0:CH, qs],
                    start=True, stop=True)
                nc.tensor.matmul(
                    s2[:, 512:1024], kt[CH:P, off:off + P], q_sb[m][CH:P, qs],
                    start=True, stop=True, tile_position=(64, 0))
                ex = epl.tile([P, TH], bf16, tag="expS", name=f"ex{m}_{s}_{tb}")
                nc.scalar.activation(
                    out=ex, in_=s2, func=AF.Exp,
                    bias=pad_sb[m][:, s:s + 1], scale=0.125)
                return ex

            def emit_pv_den(m, s, tb, ex, o2, den, first, last):
                ts = slice(tb * 512, (tb + 1) * 512)
                nc.tensor.matmul(
                    o2[0:CH, ts], vhat[m][s][:, 0:CH], ex[:, 0:512],
                    start=first, stop=last)
                nc.tensor.matmul(
                    o2[CH:P, ts], vhat[m][s][:, CH:P], ex[:, 512:1024],
                    start=first, stop=last, tile_position=(0, 64))
                nc.tensor.matmul(
                    den[0:1, ts], ones128, ex[:, 0:512],
                    start=first, stop=last)
                nc.tensor.matmul(
                    den[64:65, ts], ones128, ex[:, 512:1024],
                    start=first, stop=last, tile_position=(0, 64))

            def emit_recip(m, den):
                # custom-DVE ops silently no-op at base partition 64, so run
                # one op over [0:65] from base 0; rows 1:63 are memset filler.
                nc.vector.reciprocal_approx_fast(
                    out=rc[m][0:65, :], in_=den[0:65, :])

            def emit_bcast_mul(m, o2):
                # broadcast 1/l_a -> partitions 0:64, 1/l_b -> 64:128, then
                # h2 = O2 * bc in one [128, TH] DVE multiply.
                bc2 = mmp.tile([P, TH], f32, tag="mm", name=f"bc2_{m}")
                for tb in range(2):
                    ts = slice(tb * 512, (tb + 1) * 512)
                    nc.tensor.matmul(
                        bc2[:, ts], sel, rc[m][:, ts],
                        start=True, stop=True)
                nc.vector.tensor_copy(out=bc_sb[m], in_=bc2)
                nc.vector.tensor_mul(h2[m], o2[0:P, :], bc_sb[m])

            # ---------- schedule ----------
            emit_pe_warmup()
            emit_k_block(0, 0)
            emit_q(0)
            emit_k_block(0, 1)
            emit_k_block(0, 2)
            for s in range(sc_n):
                emit_v_chunk(0, s)
            emit_norm_memsets()
            dummy = None

            # Global chunk stream: S/exp for all pairs runs back-to-back so
            # the ACT engine never starves; PV/den pops lag by LAG chunks and
            # pause HOLD extra chunks at each pair switch so the previous
            # pair's normalize chain (recip -> bc2 -> mul) finishes before the
            # next pair needs the single O2/den PSUM buffer.
            LAG = 5
            HOLD = 4
            o2_t = {}
            den_t = {}
            stream = [(m, s, tb) for m in range(NMG)
                      for s in range(sc_n) for tb in range(2)]
            # filler work (v/qk/gn of later pairs) spread across earlier pairs
            per_pair = 2 * sc_n
            fill_at = {}
            for m in range(NMG):
                fillers = []
                if m == 0:
                    fillers += [("q", 1), ("kb", 1, 0), ("kb", 1, 1),
                                ("kb", 1, 2)]
                if m + 1 < NMG:
                    fillers += [("v", m + 1, s) for s in range(sc_n)]
                if m == 0:
                    fillers += [("q", 2), ("kb", 2, 0), ("kb", 2, 1),
                                ("kb", 2, 2)]
                if m == 1:
                    fillers += [("q", 3), ("kb", 3, 0), ("kb", 3, 1),
                                ("kb", 3, 2)]
                if fillers:
                    step = max(1, per_pair // len(fillers))
                    for idx, f in enumerate(fillers):
                        gi = m * per_pair + min(idx * step + 1, per_pair - 1)
                        fill_at.setdefault(gi, []).append(f)

            q = []
            hold = 0
            pend_norm = None   # (countdown, m, o2)

            def pop_one():
                nonlocal pend_norm
                m_, s_, tb_, ex_ = q.pop(0)
                if (s_, tb_) == (0, 0):
                    # previous pair's normalize MUST precede this pair's
                    # first PV (single O2/den buffer) in every queue
                    if pend_norm is not None:
                        emit_bcast_mul(pend_norm[1], pend_norm[2])
                        pend_norm = None
                    o2_t[m_] = o2p.tile([P, TH], f32, tag="O2",
                                        name=f"o2_{m_}")
                    den_t[m_] = denp.tile([65, TH], f32, tag="den",
                                          name=f"den{m_}")
                    nc.vector.memset(den_t[m_][0:64, :], 1.0)
                emit_pv_den(m_, s_, tb_, ex_, o2_t[m_], den_t[m_],
                            s_ == 0, s_ == sc_n - 1)
                if (s_, tb_) == (sc_n - 1, 1):
                    emit_recip(m_, den_t[m_])
                    pend_norm = (2, m_, o2_t[m_])
                    return True
                return False

            for gi, (m, s, tb) in enumerate(stream):
                ex = emit_s_exp(m, s, tb)
                q.append((m, s, tb, ex))
                if pend_norm is not None:
                    cd, m_, o2_ = pend_norm
                    if cd == 0:
                        emit_bcast_mul(m_, o2_)
                        pend_norm = None
                    else:
                        pend_norm = (cd - 1, m_, o2_)
                for f in fill_at.get(gi, []):
                    if f[0] == "v":
                        emit_v_chunk(f[1], f[2])
                    elif f[0] == "q":
                        emit_q(f[1])
                    elif f[0] == "kb":
                        emit_k_block(f[1], f[2])
                if hold > 0:
                    hold -= 1
                else:
                    lag_now = LAG if m < NMG - 1 else 3
                    budget = 2 if len(q) > lag_now + 2 else 1
                    while budget > 0 and len(q) > lag_now:
                        budget -= 1
                        if pop_one():
                            hold = HOLD
                            break
            while q:
                if pend_norm is not None and pend_norm[0] == 0:
                    emit_bcast_mul(pend_norm[1], pend_norm[2])
                    pend_norm = None
                elif pend_norm is not None:
                    pend_norm = (pend_norm[0] - 1, pend_norm[1], pend_norm[2])
                pop_one()
            # pairs 0-2 proj partials overlap pair-3's normalize chain
            pp_sb = []
            for ci in range(4):
                t = mmp.tile([P, TH], f32, tag="mm", name=f"p012_{ci}")
                for tb in range(2):
                    for mp in range(3):
                        nc.tensor.matmul(
                            t[:, tb * 512:(tb + 1) * 512],
                            wp2[mp][:, ci * P:(ci + 1) * P],
                            h2[mp][:, tb * 512:(tb + 1) * 512],
                            start=(mp == 0), stop=(mp == 2))
                nc.vector.tensor_add(psb_t[ci], t, xres[ci])
                pp_sb.append(psb_t[ci])
                if ci == 1 and pend_norm is not None:
                    emit_bcast_mul(pend_norm[1], pend_norm[2])
                    pend_norm = None
            if pend_norm is not None:
                emit_bcast_mul(pend_norm[1], pend_norm[2])
            outq = [nc.sync, nc.scalar, nc.gpsimd, nc.sync]
            for ci in range(4):
                pp = mmp.tile([P, TH], f32, tag="mm", name=f"pp3_{ci}")
                for tb in range(2):
                    nc.tensor.matmul(
                        pp[:, tb * 512:(tb + 1) * 512],
                        wp2[3][:, ci * P:(ci + 1) * P],
                        h2[3][:, tb * 512:(tb + 1) * 512],
                        start=True, stop=True)
                ot = msc.tile([P, TH], bf16, tag="out", name=f"ot{ci}")
                nc.vector.tensor_add(ot, pp, pp_sb[ci])
                outq[ci].dma_start(d_out[ci * P:(ci + 1) * P, :], ot)

    nc.finalize()
    return nc


def _prepare(inputs):
    """Host-side shard preparation. Returns (nkv, in_maps)."""
    x = np.asarray(inputs["x"], dtype=np.float32)
    pos = np.asarray(inputs["pos"], dtype=np.float32)
    mask = np.asarray(inputs["mask"])
    gn_w = np.asarray(inputs["gn_w"], dtype=np.float32)
    gn_b = np.asarray(inputs["gn_b"], dtype=np.float32)
    qkv_w = np.asarray(inputs["qkv_w"], dtype=np.float32)
    qkv_b = np.asarray(inputs["qkv_b"], dtype=np.float32)
    proj_w = np.asarray(inputs["proj_w"], dtype=np.float32)
    proj_b = np.asarray(inputs["proj_b"], dtype=np.float32)

    # GroupNorm folded to per-channel affine per batch (stats over full T,
    # matching the reference exactly).
    xg = x.reshape(B, NUM_GROUPS, GS, T)
    mu = xg.mean(axis=(2, 3))
    var = xg.var(axis=(2, 3))
    rs = 1.0 / np.sqrt(var + EPS)
    rs_c = np.repeat(rs, GS, axis=1)
    mu_c = np.repeat(mu, GS, axis=1)
    A_all = rs_c * gn_w[None, :]
    B_all = gn_b[None, :] - mu_c * A_all

    # reorder qkv weights: reference splits rows as [h, (q|k|v), 64]; we
    # additionally permute heads into slot order PERM.
    perm = np.asarray(PERM)
    w3 = qkv_w.reshape(H, 3, CH, C)
    wq_r = w3[perm, 0].reshape(C, C)
    wk_r = w3[perm, 1].reshape(C, C)
    wv_r = w3[perm, 2].reshape(C, C)
    wqkv_r = np.concatenate([wq_r, wk_r, wv_r], axis=0)  # [3C, C] slot order
    # proj: input channels permuted to slot order
    perm_idx = (perm[:, None] * CH + np.arange(CH)[None, :]).reshape(-1)
    wpT = np.ascontiguousarray(proj_w.T[perm_idx]).astype(BF16)

    # per mask-group key compaction (mask quirk: group m uses mask[m])
    keep = [np.flatnonzero(~mask[m, 0]) for m in range(NMG)]
    n_max = max(max(len(kp) for kp in keep), 1)
    nkv = ((n_max + P - 1) // P) * P

    x_kv_all = []      # per batch: [NMG, C, nkv]
    for bb in range(B):
        xkv_b = np.zeros((NMG, C, nkv), dtype=BF16)
        for m in range(NMG):
            kp = keep[m]
            xkv_b[m, :, :len(kp)] = x[bb][:, kp]
        x_kv_all.append(xkv_b)

    sc_n = nkv // P
    pad = np.zeros((NMG, nkv), dtype=np.float32)
    for m in range(NMG):
        pad[m, len(keep[m]):] = -1e9
    # device layout [NMG, P, sc_n]: pad2[m, p, s] = pad[m, s*128+p]
    pad2 = np.ascontiguousarray(
        pad.reshape(NMG, sc_n, P).transpose(0, 2, 1))

    in_maps = []
    for core in range(8):
        bb, half = core // 2, core % 2
        ts = slice(half * TH, (half + 1) * TH)
        posb = pos[bb * H:(bb + 1) * H]        # [8, 64, 2048] true head order

        x_q = np.ascontiguousarray(x[bb][:, ts]).astype(BF16)
        x_res = np.ascontiguousarray(
            x[bb][:, ts] + proj_b[:, None]).astype(BF16)
        # GroupNorm folded into the qkv weights/biases for this batch:
        # W @ (x*A + B) + b = (W*A) @ x + (W @ B + b)
        wqkv_eff = wqkv_r * A_all[bb][None, :]
        b3 = qkv_b.reshape(H, 3, CH)
        b_eff = (np.concatenate([b3[perm, 0].reshape(C), b3[perm, 1].reshape(C),
                                 b3[perm, 2].reshape(C)])
                 + wqkv_r @ B_all[bb])
        wqkvT = np.ascontiguousarray(wqkv_eff.T).astype(BF16)
        bq = b_eff[0:C]
        bk = b_eff[C:2 * C]
        bv = b_eff[2 * C:3 * C]
        pos_q = (posb[perm][:, :, ts].reshape(C, TH) + bq[:, None]).astype(BF16)

        pos_kv = np.zeros((NMG, P, nkv), dtype=BF16)
        posT = np.zeros((NMG, nkv, P), dtype=np.float32)
        sc_n = nkv // P
        for m in range(NMG):
            kp = keep[m]
            nb = len(kp)
            for j, hh in enumerate((m, m + 4)):   # slots 2m, 2m+1
                sl = slice((2 * m + j) * CH, (2 * m + j + 1) * CH)
                pos_kv[m, j * CH:(j + 1) * CH, :nb] = (
                    posb[hh][:, kp] + bk[sl][:, None])
                posT[m, :nb, j * CH:(j + 1) * CH] = (
                    posb[hh][:, kp].T + bv[sl][None, :])

        # posT device layout [NMG, P, sc_n*P]: [m, p, s*128+c] = posT[m, s*128+p, c]
        posT2 = np.ascontiguousarray(
            posT.reshape(NMG, sc_n, P, P).transpose(0, 2, 1, 3)
            .reshape(NMG, P, sc_n * P))
        in_maps.append({
            "x_q": x_q,
            "x_kv": x_kv_all[bb],
            "x_res": x_res,
            "pos_q": pos_q,
            "pos_kv": pos_kv,
            "posT_kv": posT2.astype(BF16),
            "wqkvT": wqkvT,
            "wpT": wpT,
            "pad_bias": pad2,
        })
    return nkv, in_maps


def kernel(**inputs):
    from concourse.bass_utils import run_bass_kernel_spmd

    nkv, in_maps = _prepare(inputs)
    if nkv not in _graph_cache:
        _graph_cache[nkv] = _build(nkv)
    nc = _graph_cache[nkv]

    res = run_bass_kernel_spmd(nc, in_maps, core_ids=list(range(8)))
    results = res.results

    out = np.empty((B, C, T), dtype=np.float32)
    for core in range(8):
        bb, half = core // 2, core % 2
        out[bb][:, half * TH:(half + 1) * TH] = np.asarray(results[core]["out"], dtype=np.float32)
    return out


# revision 41
# speedup vs baseline: 1.0424x; 1.0424x over previous
"""Trainium2 Bass kernel for nn_AttentionBlock (sparse attention block), v2.

Reference computation (B=4, C=512, T=2048, H=8 heads, 32 GN groups):
    xn  = GroupNorm(x) * gn_w + gn_b
    qkv = qkv_w @ xn + qkv_b            (1x1 conv)
    q,k,v = split(reshape(qkv, [B*H, 192, T])) ; each += pos
    S   = (q*s)^T (k*s),  s = ch^-0.25  => scale 1/8 on logits
    S[mask keys] = -1e9 ; P = softmax(S, axis=keys)
    h   = P @ v ; out = x + proj_w @ h + proj_b

Mask quirk (faithful to the reference): jnp.tile(mask,(H,1,1)) tiles
head-major, so attention row n = b*H + h uses mask[n % B] = mask[h % 4].

Sharding: 8 cores = (batch b, query-half j).  Each core computes
out[b][:, j*1024:(j+1)*1024] completely; host concatenates.  No collectives.

Sparsity: host compacts the key axis per mask-group m = h%4 with
keep_m = ~mask[m] (about half of T), padded to a multiple of 128.  Padded
key rows get an exp-bias of -1e9 so their softmax weight is exactly 0.

v2 device layout (head pair (a,b) = slots (2m, 2m+1) = true heads (m, m+4)):
  - S^T [keys, queries] computed per (m, s-chunk, tb): two K=64 matmuls
    row-tiled at array rows 0:64 / 64:128 into one [128,1024] PSUM tile
    (head a cols 0:512, head b cols 512:1024) -> ONE exp per chunk.
  - PV col-tiled: head a -> O2[0:64], head b -> O2[64:128] of a single
    [128,TH] accumulator; softmax denominators from two M=1 matmuls
    (ones lhsT) into den[0] / den[64], re-streaming the same ex chunk.
  - normalize: reciprocal_approx_fast on the two den rows, then ONE
    K=65 f32r select-matmul broadcasts 1/l_a to partitions 0:64 and
    1/l_b to 64:128; one DVE multiply forms h2 [128, TH] per pair.
  - proj contracts pairs with K=128 (wp row-pairs vs h2 tiles).
  - Emission is software-pipelined: PV/den lag S/exp by several chunks,
    v-phase and qk of later pairs interleave as PE filler, so the PE
    never idles long enough for the HAM clock gate to re-throttle.
PSUM budget (8 banks): mm pool 2x[128,1024] (4) + O2 (2) + den (2).
"""

import numpy as np
import ml_dtypes

B, C, T, H = 4, 512, 2048, 8
CH = C // H          # 64 channels per head
TH = T // 2          # 1024 query columns per core
P = 128
NUM_GROUPS = 32
GS = C // NUM_GROUPS  # 16 channels per group
EPS = 1e-5
BF16 = ml_dtypes.bfloat16
NMG = 4              # mask groups (= B); group m covers heads m and m+4
PERM = [0, 4, 1, 5, 2, 6, 3, 7]  # slot s holds true head PERM[s]

_graph_cache = {}


def _build(nkv):
    """Build the Bass graph for one core (SPMD: all 8 cores run this graph)."""
    import concourse.tile as tile
    from concourse import bacc, mybir

    f32 = mybir.dt.float32
    f32r = mybir.dt.float32r
    bf16 = mybir.dt.bfloat16
    AF = mybir.ActivationFunctionType
    OP = mybir.AluOpType

    sc_n = nkv // P  # number of 128-wide key chunks

    nc = bacc.Bacc("TRN2")

    # ---- DRAM parameters (per-core shards; host fills these) ----
    d_xq = nc.dram_tensor("x_q", [C, TH], bf16, kind="ExternalInput")
    d_xkv = nc.dram_tensor("x_kv", [NMG, C, nkv], bf16, kind="ExternalInput")
    d_xres = nc.dram_tensor("x_res", [C, TH], bf16, kind="ExternalInput")
    d_posq = nc.dram_tensor("pos_q", [C, TH], bf16, kind="ExternalInput")
    d_poskv = nc.dram_tensor("pos_kv", [NMG, P, nkv], bf16, kind="ExternalInput")
    d_posT = nc.dram_tensor("posT_kv", [NMG, P, nkv], bf16, kind="ExternalInput")
    d_wqkvT = nc.dram_tensor("wqkvT", [C, 3 * C], bf16, kind="ExternalInput")
    d_wpT = nc.dram_tensor("wpT", [C, C], bf16, kind="ExternalInput")
    d_pad = nc.dram_tensor("pad_bias", [NMG, P, nkv // P], f32,
                            kind="ExternalInput")
    d_out = nc.dram_tensor("out", [C, TH], bf16, kind="ExternalOutput")
    DBG = False
    if DBG:
        d_den = nc.dram_tensor("dbg_den", [NMG, 2, TH], f32,
                               kind="ExternalOutput")
        d_rc = nc.dram_tensor("dbg_rc", [NMG, 2, TH], f32,
                              kind="ExternalOutput")
        d_h2 = nc.dram_tensor("dbg_h2", [NMG, P, TH], f32,
                              kind="ExternalOutput")
        d_o2 = nc.dram_tensor("dbg_o2", [NMG, P, TH], f32,
                              kind="ExternalOutput")
        d_ex0 = nc.dram_tensor("dbg_ex0", [P, TH], f32, kind="ExternalOutput")

    with tile.TileContext(nc) as tc, \
         tc.tile_pool(name="persist", bufs=1) as pers:

        def ptile(shape, dt_, name):
            return pers.tile(shape, dt_, tag=name, name=name)

        # --- tiny exp to pull the ACT table load off the critical path ---
        warm_in = ptile([1, 1], f32, "warm_in")
        warm_out = ptile([1, 1], f32, "warm_out")
        nc.vector.memset(warm_in, 0.0)
        nc.scalar.activation(out=warm_out, in_=warm_in, func=AF.Exp)

        # --- persistent SBUF arrays ---
        xq = [ptile([P, TH], bf16, f"xq{i}") for i in range(4)]
        xkv = [[ptile([P, nkv], bf16, f"xkv{m}_{i}") for i in range(4)]
               for m in range(NMG)]
        wq = [ptile([P, 3 * C], bf16, f"wq{i}") for i in range(4)]
        wp2 = [ptile([P, C], bf16, f"wp{m}") for m in range(NMG)]
        posq = [ptile([P, TH], bf16, f"posq{i}") for i in range(4)]
        poskv = [ptile([P, nkv], bf16, f"poskv{m}") for m in range(NMG)]
        q_sb = [ptile([P, TH], bf16, f"q{m}") for m in range(NMG)]
        kblk = [(st, min(512, nkv - st)) for st in range(0, nkv, 512)]
        k_sb = [[ptile([P, w], bf16, f"k{m}_{bi}")
                 for bi, (st, w) in enumerate(kblk)] for m in range(NMG)]
        vhat = [[ptile([P, P], bf16, f"vhat{m}_{s}") for s in range(sc_n)]
                for m in range(NMG)]
        h2 = [ptile([P, TH], bf16, f"h2_{m}") for m in range(NMG)]
        xres = [ptile([P, TH], bf16, f"xres{i}") for i in range(4)]
        pad_sb = [ptile([P, sc_n], f32, f"pad{m}") for m in range(NMG)]
        # select matrix for the paired reciprocal broadcast (K=65 matmul):
        # row 0 -> output partitions 0:64, row 64 -> partitions 64:128.
        sel = ptile([65, P], f32, "sel")
        ones128 = ptile([P, 1], bf16, "ones128")
        # per-pair reciprocal rows (rows 1:64 stay 1.0 = harmless filler)
        rc = [ptile([65, TH], f32, f"rc{m}") for m in range(NMG)]
        bc_sb = [ptile([P, TH], bf16, f"bcs{m}") for m in range(NMG)]
        psb_t = [ptile([P, TH], f32, f"psb{ci}") for ci in range(4)]

        wrm = ptile([P, 512], bf16, "wrm")

        def emit_pe_warmup():
            nc.vector.memset(ones128, 1.0)
            nc.vector.memset(wrm, 0.0)
            wps = None
            for w_i in range(36):
                if w_i % 12 == 0:
                    wps = mmp.tile([1, 512], f32, tag="mm", name=f"wps{w_i}")
                nc.tensor.matmul(wps, ones128, wrm, start=True, stop=True)

        def emit_norm_memsets():
            nc.vector.memset(sel, 0.0)
            nc.vector.memset(sel[0:1, 0:CH], 1.0)
            nc.vector.memset(sel[64:65, CH:P], 1.0)
            nc.vector.memset(ones128, 1.0)
            for m in range(NMG):
                nc.vector.memset(rc[m], 1.0)

        # --- input DMAs, grouped by consuming pair and spread over idle
        # engine queues (each trigger serializes ~0.7us on its queue) ---
        posvT = [ptile([P, sc_n * P], bf16, f"pvT{m}") for m in range(NMG)]

        def emit_pair_dmas(q, m):
            for i in range(4):
                q.dma_start(xkv[m][i], d_xkv[m, i * P:(i + 1) * P, :])
            q.dma_start(poskv[m], d_poskv[m, :, :])
            q.dma_start(pad_sb[m], d_pad[m, :, :])
            q.dma_start(posvT[m], d_posT[m, :, :])

        # pair-0-critical data: (xkv0[i], wq[i]) pairs split across the
        # three DMA queues so v(0)'s accumulation matmuls start as soon as
        # each contraction slice lands
        nc.sync.dma_start(xkv[0][0], d_xkv[0, 0:P, :])
        nc.sync.dma_start(wq[0], d_wqkvT[0:P, :])
        nc.sync.dma_start(pad_sb[0], d_pad[0, :, :])
        nc.sync.dma_start(poskv[0], d_poskv[0, :, :])
        nc.sync.dma_start(posq[0], d_posq[0:P, :])
        nc.sync.dma_start(posq[1], d_posq[P:2 * P, :])
        nc.scalar.dma_start(xkv[0][1], d_xkv[0, P:2 * P, :])
        nc.scalar.dma_start(wq[1], d_wqkvT[P:2 * P, :])
        nc.scalar.dma_start(wq[3], d_wqkvT[3 * P:4 * P, :])
        nc.scalar.dma_start(posvT[0], d_posT[0, :, :])
        nc.scalar.dma_start(posq[2], d_posq[2 * P:3 * P, :])
        nc.scalar.dma_start(posq[3], d_posq[3 * P:4 * P, :])
        nc.gpsimd.dma_start(xkv[0][2], d_xkv[0, 2 * P:3 * P, :])
        nc.gpsimd.dma_start(wq[2], d_wqkvT[2 * P:3 * P, :])
        nc.gpsimd.dma_start(xkv[0][3], d_xkv[0, 3 * P:4 * P, :])
        for i in range(4):
            nc.gpsimd.dma_start(xq[i], d_xq[i * P:(i + 1) * P, :])
        emit_pair_dmas(nc.scalar, 1)
        emit_pair_dmas(nc.sync, 2)
        emit_pair_dmas(nc.gpsimd, 3)
        for m in range(NMG):
            nc.scalar.dma_start(wp2[m], d_wpT[m * P:(m + 1) * P, :])
        for i in range(4):
            nc.sync.dma_start(xres[i], d_xres[i * P:(i + 1) * P, :])

        with tc.tile_pool(name="mm", bufs=2, space="PSUM") as mmp, \
             tc.tile_pool(name="o2p", bufs=1, space="PSUM") as o2p, \
             tc.tile_pool(name="denp", bufs=1, space="PSUM") as denp, \
             tc.tile_pool(name="exps", bufs=12) as epl, \
             tc.tile_pool(name="misc", bufs=2) as msc:

            def emit_v_chunk(m, s, pool=None, tag="mm"):
                # vhat[m][s] [128 keys, 128] = (xn_kv^T @ Wv_pair) + posT
                pv = (pool or mmp).tile([P, P], f32, tag=tag,
                                        name=f"psv{m}_{s}")
                for i in range(4):
                    nc.tensor.matmul(
                        pv, xkv[m][i][:, s * P:(s + 1) * P],
                        wq[i][:, 2 * C + m * P:2 * C + (m + 1) * P],
                        start=(i == 0), stop=(i == 3))
                nc.vector.tensor_add(vhat[m][s], pv,
                                     posvT[m][:, s * P:(s + 1) * P])

            def emit_q(m):
                # q channels (slot order) [128*m, 128*m+128)
                pq = mmp.tile([P, TH], f32, tag="mm", name=f"psq{m}")
                for tb in range(2):
                    for i in range(4):
                        nc.tensor.matmul(
                            pq[:, tb * 512:(tb + 1) * 512],
                            wq[i][:, m * P:(m + 1) * P],
                            xq[i][:, tb * 512:(tb + 1) * 512],
                            start=(i == 0), stop=(i == 3))
                nc.vector.tensor_add(q_sb[m], pq, posq[m])

            def emit_k_block(m, bi):
                for bj, (st, w) in enumerate(kblk):
                    if bj != bi:
                        continue
                    pk = mmp.tile([P, 512], f32, tag="mm", name=f"psk{m}_{st}")
                    for i in range(4):
                        nc.tensor.matmul(
                            pk[:, 0:w],
                            wq[i][:, C + m * P:C + (m + 1) * P],
                            xkv[m][i][:, st:st + w],
                            start=(i == 0), stop=(i == 3))
                    nc.vector.tensor_add(
                        k_sb[m][bi], pk[:, 0:w],
                        poskv[m][:, st:st + w])

            def emit_k(m):
                for bi in range(len(kblk)):
                    emit_k_block(m, bi)

            def emit_qk(m):
                emit_q(m)
                emit_k(m)

            # ---------- attention pipeline ----------
            def emit_s_exp(m, s, tb):
                """S^T chunk for both heads + one fused exp."""
                s2 = mmp.tile([P, TH], f32, tag="mm", name=f"s2_{m}_{s}_{tb}")
                qs = slice(tb * 512, (tb + 1) * 512)
                bi, off = (s * P) // 512, (s * P) % 512
                kt = k_sb[m][bi]
                nc.tensor.matmul(
                    s2[:, 0:512], kt[0:CH, off:off + P], q_sb[m][0:CH, qs],
                    start=True, stop=True)
                nc.tensor.matmul(
                    s2[:, 512:1024], kt[CH:P, off:off + P], q_sb[m][CH:P, qs],
                    start=True, stop=True, tile_position=(64, 0))
                ex = epl.tile([P, TH], bf16, tag="expS", name=f"ex{m}_{s}_{tb}")
                nc.scalar.activation(
                    out=ex, in_=s2, func=AF.Exp,
                    bias=pad_sb[m][:, s:s + 1], scale=0.125)
                return ex

            def emit_pv_den(m, s, tb, ex, o2, den, first, last):
                ts = slice(tb * 512, (tb + 1) * 512)
                nc.tensor.matmul(
                    o2[0:CH, ts], vhat[m][s][:, 0:CH], ex[:, 0:512],
                    start=first, stop=last)
                nc.tensor.matmul(
                    o2[CH:P, ts], vhat[m][s][:, CH:P], ex[:, 512:1024],
                    start=first, stop=last, tile_position=(0, 64))
                nc.tensor.matmul(
                    den[0:1, ts], ones128, ex[:, 0:512],
                    start=first, stop=last)
                nc.tensor.matmul(
                    den[64:65, ts], ones128, ex[:, 512:1024],
                    start=first, stop=last, tile_position=(0, 64))

            def emit_recip(m, den):
                # custom-DVE ops silently no-op at base partition 64, so run
                # one op over [0:65] from base 0; rows 1:63 are memset filler.
                nc.vector.reciprocal_approx_fast(
                    out=rc[m][0:65, :], in_=den[0:65, :])

            def emit_bcast_mul(m, o2):
                # broadcast 1/l_a -> partitions 0:64, 1/l_b -> 64:128, then
                # h2 = O2 * bc in one [128, TH] DVE multiply.
                bc2 = mmp.tile([P, TH], f32, tag="mm", name=f"bc2_{m}")
                for tb in range(2):
                    ts = slice(tb * 512, (tb + 1) * 512)
                    nc.tensor.matmul(
                        bc2[:, ts], sel, rc[m][:, ts],
                        start=True, stop=True)
                nc.vector.tensor_copy(out=bc_sb[m], in_=bc2)
                nc.vector.tensor_mul(h2[m], o2[0:P, :], bc_sb[m])

            # ---------- schedule ----------
            emit_pe_warmup()
            for s in range(sc_n):
                emit_v_chunk(0, s)
            emit_qk(0)
            emit_qk(1)
            emit_norm_memsets()

            # Global chunk stream: S/exp for all pairs runs back-to-back so
            # the ACT engine never starves; PV/den pops lag by LAG chunks and
            # pause HOLD extra chunks at each pair switch so the previous
            # pair's normalize chain (recip -> bc2 -> mul) finishes before the
            # next pair needs the single O2/den PSUM buffer.
            LAG = 5
            HOLD = 4
            o2_t = {}
            den_t = {}
            stream = [(m, s, tb) for m in range(NMG)
                      for s in range(sc_n) for tb in range(2)]
            # filler work (v/qk/gn of later pairs) spread across earlier pairs
            per_pair = 2 * sc_n
            fill_at = {}
            for m in range(NMG):
                fillers = []
                if m + 1 < NMG:
                    fillers += [("v", m + 1, s) for s in range(sc_n)]
                if m == 0:
                    fillers += [("qk", 2)]
                if m == 1:
                    fillers += [("qk", 3)]
                if fillers:
                    step = max(1, per_pair // len(fillers))
                    for idx, f in enumerate(fillers):
                        gi = m * per_pair + min(idx * step + 2, per_pair - 1)
                        fill_at.setdefault(gi, []).append(f)

            q = []
            hold = 0
            pend_norm = None   # (countdown, m, o2)

            def pop_one():
                nonlocal pend_norm
                m_, s_, tb_, ex_ = q.pop(0)
                if (s_, tb_) == (0, 0):
                    # previous pair's normalize MUST precede this pair's
                    # first PV (single O2/den buffer) in every queue
                    if pend_norm is not None:
                        emit_bcast_mul(pend_norm[1], pend_norm[2])
                        pend_norm = None
                    o2_t[m_] = o2p.tile([P, TH], f32, tag="O2",
                                        name=f"o2_{m_}")
                    den_t[m_] = denp.tile([65, TH], f32, tag="den",
                                          name=f"den{m_}")
                    nc.vector.memset(den_t[m_][0:64, :], 1.0)
                emit_pv_den(m_, s_, tb_, ex_, o2_t[m_], den_t[m_],
                            s_ == 0, s_ == sc_n - 1)
                if (s_, tb_) == (sc_n - 1, 1):
                    emit_recip(m_, den_t[m_])
                    pend_norm = (2, m_, o2_t[m_])
                    return True
                return False

            for gi, (m, s, tb) in enumerate(stream):
                ex = emit_s_exp(m, s, tb)
                q.append((m, s, tb, ex))
                if pend_norm is not None:
                    cd, m_, o2_ = pend_norm
                    if cd == 0:
                        emit_bcast_mul(m_, o2_)
                        pend_norm = None
                    else:
                        pend_norm = (cd - 1, m_, o2_)
                for f in fill_at.get(gi, []):
                    if f[0] == "v":
                        emit_v_chunk(f[1], f[2])
                    else:
                        emit_qk(f[1])
                if hold > 0:
                    hold -= 1
                else:
                    lag_now = LAG if m < NMG - 1 else 3
                    budget = 2 if len(q) > lag_now + 2 else 1
                    while budget > 0 and len(q) > lag_now:
                        budget -= 1
                        if pop_one():
                            hold = HOLD
                            break
            while q:
                if pend_norm is not None and pend_norm[0] == 0:
                    emit_bcast_mul(pend_norm[1], pend_norm[2])
                    pend_norm = None
                elif pend_norm is not None:
                    pend_norm = (pend_norm[0] - 1, pend_norm[1], pend_norm[2])
                pop_one()
            # pairs 0-2 proj partials overlap pair-3's normalize chain
            pp_sb = []
            for ci in range(4):
                t = mmp.tile([P, TH], f32, tag="mm", name=f"p012_{ci}")
                for tb in range(2):
                    for mp in range(3):
                        nc.tensor.matmul(
                            t[:, tb * 512:(tb + 1) * 512],
                            wp2[mp][:, ci * P:(ci + 1) * P],
                            h2[mp][:, tb * 512:(tb + 1) * 512],
                            start=(mp == 0), stop=(mp == 2))
                nc.vector.tensor_add(psb_t[ci], t, xres[ci])
                pp_sb.append(psb_t[ci])
                if ci == 1 and pend_norm is not None:
                    emit_bcast_mul(pend_norm[1], pend_norm[2])
                    pend_norm = None
            if pend_norm is not None:
                emit_bcast_mul(pend_norm[1], pend_norm[2])
            outq = [nc.sync, nc.scalar, nc.gpsimd, nc.sync]
            for ci in range(4):
                pp = mmp.tile([P, TH], f32, tag="mm", name=f"pp3_{ci}")
                for tb in range(2):
                    nc.tensor.matmul(
                        pp[:, tb * 512:(tb + 1) * 512],
                        wp2[3][:, ci * P:(ci + 1) * P],
                        h2[3][:, tb * 512:(tb + 1) * 512],
                        start=True, stop=True)
                ot = msc.tile([P, TH], bf16, tag="out", name=f"ot{ci}")
                nc.vector.tensor_add(ot, pp, pp_sb[ci])
                outq[ci].dma_start(d_out[ci * P:(ci + 1) * P, :], ot)

    nc.finalize()
    return nc


def _prepare(inputs):
    """Host-side shard preparation. Returns (nkv, in_maps)."""
    x = np.asarray(inputs["x"], dtype=np.float32)
    pos = np.asarray(inputs["pos"], dtype=np.float32)
    mask = np.asarray(inputs["mask"])
    gn_w = np.asarray(inputs["gn_w"], dtype=np.float32)
    gn_b = np.asarray(inputs["gn_b"], dtype=np.float32)
    qkv_w = np.asarray(inputs["qkv_w"], dtype=np.float32)
    qkv_b = np.asarray(inputs["qkv_b"], dtype=np.float32)
    proj_w = np.asarray(inputs["proj_w"], dtype=np.float32)
    proj_b = np.asarray(inputs["proj_b"], dtype=np.float32)

    # GroupNorm folded to per-channel affine per batch (stats over full T,
    # matching the reference exactly).
    xg = x.reshape(B, NUM_GROUPS, GS, T)
    mu = xg.mean(axis=(2, 3))
    var = xg.var(axis=(2, 3))
    rs = 1.0 / np.sqrt(var + EPS)
    rs_c = np.repeat(rs, GS, axis=1)
    mu_c = np.repeat(mu, GS, axis=1)
    A_all = rs_c * gn_w[None, :]
    B_all = gn_b[None, :] - mu_c * A_all

    # reorder qkv weights: reference splits rows as [h, (q|k|v), 64]; we
    # additionally permute heads into slot order PERM.
    perm = np.asarray(PERM)
    w3 = qkv_w.reshape(H, 3, CH, C)
    wq_r = w3[perm, 0].reshape(C, C)
    wk_r = w3[perm, 1].reshape(C, C)
    wv_r = w3[perm, 2].reshape(C, C)
    wqkv_r = np.concatenate([wq_r, wk_r, wv_r], axis=0)  # [3C, C] slot order
    # proj: input channels permuted to slot order
    perm_idx = (perm[:, None] * CH + np.arange(CH)[None, :]).reshape(-1)
    wpT = np.ascontiguousarray(proj_w.T[perm_idx]).astype(BF16)

    # per mask-group key compaction (mask quirk: group m uses mask[m])
    keep = [np.flatnonzero(~mask[m, 0]) for m in range(NMG)]
    n_max = max(max(len(kp) for kp in keep), 1)
    nkv = ((n_max + P - 1) // P) * P

    x_kv_all = []      # per batch: [NMG, C, nkv]
    for bb in range(B):
        xkv_b = np.zeros((NMG, C, nkv), dtype=BF16)
        for m in range(NMG):
            kp = keep[m]
            xkv_b[m, :, :len(kp)] = x[bb][:, kp]
        x_kv_all.append(xkv_b)

    sc_n = nkv // P
    pad = np.zeros((NMG, nkv), dtype=np.float32)
    for m in range(NMG):
        pad[m, len(keep[m]):] = -1e9
    # device layout [NMG, P, sc_n]: pad2[m, p, s] = pad[m, s*128+p]
    pad2 = np.ascontiguousarray(
        pad.reshape(NMG, sc_n, P).transpose(0, 2, 1))

    in_maps = []
    for core in range(8):
        bb, half = core // 2, core % 2
        ts = slice(half * TH, (half + 1) * TH)
        posb = pos[bb * H:(bb + 1) * H]        # [8, 64, 2048] true head order

        x_q = np.ascontiguousarray(x[bb][:, ts]).astype(BF16)
        x_res = np.ascontiguousarray(
            x[bb][:, ts] + proj_b[:, None]).astype(BF16)
        # GroupNorm folded into the qkv weights/biases for this batch:
        # W @ (x*A + B) + b = (W*A) @ x + (W @ B + b)
        wqkv_eff = wqkv_r * A_all[bb][None, :]
        b3 = qkv_b.reshape(H, 3, CH)
        b_eff = (np.concatenate([b3[perm, 0].reshape(C), b3[perm, 1].reshape(C),
                                 b3[perm, 2].reshape(C)])
                 + wqkv_r @ B_all[bb])
        wqkvT = np.ascontiguousarray(wqkv_eff.T).astype(BF16)
        bq = b_eff[0:C]
        bk = b_eff[C:2 * C]
        bv = b_eff[2 * C:3 * C]
        pos_q = (posb[perm][:, :, ts].reshape(C, TH) + bq[:, None]).astype(BF16)

        pos_kv = np.zeros((NMG, P, nkv), dtype=BF16)
        posT = np.zeros((NMG, nkv, P), dtype=np.float32)
        sc_n = nkv // P
        for m in range(NMG):
            kp = keep[m]
            nb = len(kp)
            for j, hh in enumerate((m, m + 4)):   # slots 2m, 2m+1
                sl = slice((2 * m + j) * CH, (2 * m + j + 1) * CH)
                pos_kv[m, j * CH:(j + 1) * CH, :nb] = (
                    posb[hh][:, kp] + bk[sl][:, None])
                posT[m, :nb, j * CH:(j + 1) * CH] = (
                    posb[hh][:, kp].T + bv[sl][None, :])

        # posT device layout [NMG, P, sc_n*P]: [m, p, s*128+c] = posT[m, s*128+p, c]
        posT2 = np.ascontiguousarray(
            posT.reshape(NMG, sc_n, P, P).transpose(0, 2, 1, 3)
            .reshape(NMG, P, sc_n * P))
        in_maps.append({
            "x_q": x_q,
            "x_kv": x_kv_all[bb],
            "x_res": x_res,
            "pos_q": pos_q,
            "pos_kv": pos_kv,
            "posT_kv": posT2.astype(BF16),
            "wqkvT": wqkvT,
            "wpT": wpT,
            "pad_bias": pad2,
        })
    return nkv, in_maps


def kernel(**inputs):
    from concourse.bass_utils import run_bass_kernel_spmd

    nkv, in_maps = _prepare(inputs)
    if nkv not in _graph_cache:
        _graph_cache[nkv] = _build(nkv)
    nc = _graph_cache[nkv]

    res = run_bass_kernel_spmd(nc, in_maps, core_ids=list(range(8)))
    results = res.results

    out = np.empty((B, C, T), dtype=np.float32)
    for core in range(8):
        bb, half = core // 2, core % 2
        out[bb][:, half * TH:(half + 1) * TH] = np.asarray(results[core]["out"], dtype=np.float32)
    return out
